# revision 32
# baseline (speedup 1.0000x reference)
"""KeepTopK kernel for Trainium2.

out[i, j] = x[i, j] if x[i, j] is among the top-8 of row i else 1e6.

Strategy (pure data parallel, 8 cores, 32768 rows each):
  per [128, 2048] block (1024 rows, 8 rows per partition):
    load  : 1MB SWDGE loads (nc.gpsimd) — SWDGE completions use the
            DMASW semaphore lanes, disjoint from the stores' DMAHW
            lanes, so a slow store receipt can never false-block a
            load consumer.  Block 0 loads as 2x 512KB via HWDGE
            (nc.sync): faster first-byte than the cold Q7 SWDGE path,
            and half-granular so DVE starts ~3us earlier.
    DVE   : per 256-wide row segment: v8 = max8(x_seg) then
            y = match_replace(x, v8, BETA)   (top-8 positions -> BETA)
    ACT   : z = -y + BETA        per half-block (0 at top-8, else ~BETA)
    POOL  : o = z + x            per half-block (exact x at top-8)
    store : per half-block 512KB from SP (qSPDynamicHW).  The last
            block runs its ACT/POOL/store stages per quarter-block to
            shorten the drain chain.
match_replace replaces exactly one occurrence per top-8 element in index
order, matching jax.lax.top_k tie semantics bitwise.
"""
import numpy as np
from contextlib import ExitStack

import concourse.bass as bass
import concourse.mybir as mybir
import concourse.tile as tile
from concourse.bass_utils import run_bass_kernel_spmd

N, E, K = 262144, 256, 8
BETA = 1000000.0
NCORES = 8
ROWS_PER_CORE = N // NCORES          # 32768
ROWS_PER_PART = 8                    # rows packed per SBUF partition
BLOCK_FREE = ROWS_PER_PART * E       # 2048
ROWS_PER_BLOCK = 128 * ROWS_PER_PART  # 1024
NBLOCKS = ROWS_PER_CORE // ROWS_PER_BLOCK  # 32
HALF = BLOCK_FREE // 2               # 1024

MAX_WAITS = 1


def split_sync_waits(nc, max_waits=MAX_WAITS):
    """walrus codegen rejects instructions with more than one embedded sync
    wait; hoist extras onto same-engine NoOps placed immediately before."""
    spill_id = 0
    for f in nc.m.functions:
        for bb in f.blocks:
            insts = list(bb.instructions)
            new_insts = []
            changed = False
            for inst in insts:
                si = inst.sync_info
                waits = list(si.on_wait) if si and si.on_wait else []
                if len(waits) > max_waits:
                    extra = waits[:-max_waits]
                    si.on_wait = waits[-max_waits:]
                    for j in range(0, len(extra), max_waits):
                        nop = mybir.InstNoOp(
                            name=f"waitspill-{spill_id}", ins=[], outs=[])
                        spill_id += 1
                        nop.engine = inst.engine
                        nop.sync_info = type(si)(
                            on_wait=extra[j:j + max_waits], on_update=[])
                        new_insts.append(nop)
                    changed = True
                new_insts.append(inst)
            if changed:
                bb.instructions = new_insts


def build():
    nc = bass.Bass("TRN2", target_bir_lowering=False, debug=False)
    x = nc.dram_tensor("x", [ROWS_PER_CORE, E], mybir.dt.float32,
                       kind="ExternalInput")
    out = nc.dram_tensor("out", [ROWS_PER_CORE, E], mybir.dt.float32,
                         kind="ExternalOutput")
    xap = x.ap()
    oap = out.ap()
    f32 = mybir.dt.float32
    with tile.TileContext(nc) as tc:
        with ExitStack() as ctx:
            xpool = ctx.enter_context(tc.tile_pool(name="x", bufs=8))
            ypool = ctx.enter_context(tc.tile_pool(name="y", bufs=8))
            zpool = ctx.enter_context(tc.tile_pool(name="z", bufs=8))
            opool = ctx.enter_context(tc.tile_pool(name="o", bufs=10))
            vpool = ctx.enter_context(tc.tile_pool(name="v8", bufs=8))
            for b in range(NBLOCKS):
                r0 = b * ROWS_PER_BLOCK
                src = xap[r0:r0 + ROWS_PER_BLOCK, :].rearrange(
                    "(p r) e -> p (r e)", p=128)
                dst = oap[r0:r0 + ROWS_PER_BLOCK, :].rearrange(
                    "(p r) e -> p (r e)", p=128)
                xt = xpool.tile([128, BLOCK_FREE], f32)
                if b == 0:
                    # HWDGE half-loads: no Q7 cold start, DVE can begin
                    # on the first half while the second streams in.
                    nc.sync.dma_start(xt[:, :HALF], src[:, :HALF])
                    nc.sync.dma_start(xt[:, HALF:], src[:, HALF:])
                else:
                    nc.gpsimd.dma_start(xt[:], src)
                # epilogue granularity: quarters for the last block to
                # shorten the drain chain, halves otherwise
                nparts = 4 if b == NBLOCKS - 1 else 2
                pw = BLOCK_FREE // nparts        # columns per part
                segs = pw // E                   # segments per part
                for h in range(nparts):
                    h0 = h * pw
                    yt = ypool.tile([128, pw], f32, tag="y")
                    v8 = vpool.tile([128, 8 * segs], f32, tag="v8")
                    for s in range(segs):
                        seg = slice(h0 + s * E, h0 + (s + 1) * E)
                        nc.vector.max(v8[:, s * 8:(s + 1) * 8], xt[:, seg])
                    for s in range(segs):
                        seg = slice(h0 + s * E, h0 + (s + 1) * E)
                        nc.vector.match_replace(
                            yt[:, s * E:(s + 1) * E], v8[:, s * 8:(s + 1) * 8],
                            xt[:, seg], BETA)
                    zt = zpool.tile([128, pw], f32, tag="z")
                    nc.scalar.activation(zt[:], yt[:],
                                         mybir.ActivationFunctionType.Copy,
                                         bias=BETA, scale=-1.0)
                    ot = opool.tile([128, pw], f32, tag="o")
                    nc.gpsimd.tensor_tensor(ot[:], zt[:], xt[:, h0:h0 + pw],
                                            op=mybir.AluOpType.add)
                    nc.sync.dma_start(dst[:, h0:h0 + pw], ot[:])
    split_sync_waits(nc)
    return nc


_nc_cache = None


def _get_nc():
    global _nc_cache
    if _nc_cache is None:
        _nc_cache = build()
    return _nc_cache


def kernel(x: np.ndarray, _trace: bool = False, **_trace_kwargs):
    x = np.ascontiguousarray(np.asarray(x, dtype=np.float32))
    assert x.shape == (N, E), x.shape
    nc = _get_nc()
    in_maps = [
        {"x": x[c * ROWS_PER_CORE:(c + 1) * ROWS_PER_CORE]}
        for c in range(NCORES)
    ]
    res = run_bass_kernel_spmd(nc, in_maps, core_ids=list(range(NCORES)),
                               trace=_trace, **_trace_kwargs)
    out = np.concatenate([res.results[c]["out"] for c in range(NCORES)],
                         axis=0)
    if _trace:
        return out, res
    return out


# revision 36
# speedup vs baseline: 1.0080x; 1.0080x over previous
"""KeepTopK kernel for Trainium2.

out[i, j] = x[i, j] if x[i, j] is among the top-8 of row i else 1e6.

Strategy (pure data parallel, 8 cores, 32768 rows each):
  per [128, 2048] block (1024 rows, 8 rows per partition):
    load  : 1MB SWDGE loads (nc.gpsimd) — SWDGE completions use the
            DMASW semaphore lanes, disjoint from the stores' DMAHW
            lanes, so a slow store receipt can never false-block a
            load consumer.  Block 0 loads as 2x 512KB via HWDGE
            (nc.sync): faster first-byte than the cold Q7 SWDGE path,
            and half-granular so DVE starts ~3us earlier.
    DVE   : per 256-wide row segment: v8 = max8(x_seg) then
            y = match_replace(x, v8, BETA)   (top-8 positions -> BETA)
    ACT   : z = -y + BETA        per half-block (0 at top-8, else ~BETA)
    POOL  : o = z + x            per half-block (exact x at top-8)
    store : per half-block 512KB from SP (qSPDynamicHW).  The last
            block runs its ACT/POOL/store stages per quarter-block to
            shorten the drain chain.
match_replace replaces exactly one occurrence per top-8 element in index
order, matching jax.lax.top_k tie semantics bitwise.
"""
import numpy as np
from contextlib import ExitStack

import concourse.bass as bass
import concourse.mybir as mybir
import concourse.tile as tile
from concourse.bass_utils import run_bass_kernel_spmd

N, E, K = 262144, 256, 8
BETA = 1000000.0
NCORES = 8
ROWS_PER_CORE = N // NCORES          # 32768
ROWS_PER_PART = 8                    # rows packed per SBUF partition
BLOCK_FREE = ROWS_PER_PART * E       # 2048
ROWS_PER_BLOCK = 128 * ROWS_PER_PART  # 1024
NBLOCKS = ROWS_PER_CORE // ROWS_PER_BLOCK  # 32
HALF = BLOCK_FREE // 2               # 1024

MAX_WAITS = 1


def split_sync_waits(nc, max_waits=MAX_WAITS):
    """walrus codegen rejects instructions with more than one embedded sync
    wait; hoist extras onto same-engine NoOps placed immediately before."""
    spill_id = 0
    for f in nc.m.functions:
        for bb in f.blocks:
            insts = list(bb.instructions)
            new_insts = []
            changed = False
            for inst in insts:
                si = inst.sync_info
                waits = list(si.on_wait) if si and si.on_wait else []
                if len(waits) > max_waits:
                    extra = waits[:-max_waits]
                    si.on_wait = waits[-max_waits:]
                    for j in range(0, len(extra), max_waits):
                        nop = mybir.InstNoOp(
                            name=f"waitspill-{spill_id}", ins=[], outs=[])
                        spill_id += 1
                        nop.engine = inst.engine
                        nop.sync_info = type(si)(
                            on_wait=extra[j:j + max_waits], on_update=[])
                        new_insts.append(nop)
                    changed = True
                new_insts.append(inst)
            if changed:
                bb.instructions = new_insts


def optimize_dve_sems(nc):
    """Reduce per-instruction semaphore bookkeeping on the DVE stream.

    1. Drop DVE-on-DVE waits whose producer is >=3 instructions earlier in
       the DVE program order: the DVE pipeline is 2 deep and completes in
       order, so such waits are always satisfied at dispatch.
    2. Merge runs of [DVE-sem +1] updates into one +K on the last
       instruction of each run.  Cross-engine consumers see the increment
       at run end (later but monotonically correct).  Runs break at any
       instruction that still carries a wait, so an intra-run wait can
       never reference an increment deferred past itself.
    """
    all_insts = [i for f in nc.m.functions for bb in f.blocks
                 for i in bb.instructions]
    dve = [i for i in all_insts
           if getattr(i, "engine", None) == mybir.EngineType.DVE
           and i.sync_info is not None]

    def dve_sem_inc(inst):
        si = inst.sync_info
        ups = list(si.on_update) if si and si.on_update else []
        if len(ups) == 1 and ups[0].sync_type == "semaphore" \
                and ups[0].ant_name.startswith("DVE") \
                and ups[0].update_mode == "sem-inc" \
                and ups[0].update_value == 1:
            return ups[0]
        return None

    # cumulative original inc count after each DVE instruction
    cum = []
    c = 0
    for inst in dve:
        if dve_sem_inc(inst) is not None:
            c += 1
        cum.append(c)
    if not dve or c == 0:
        return

    def pos_of(v):
        for j, cj in enumerate(cum):
            if cj >= v:
                return j
        return len(cum) - 1

    # pass 1: drop trivially-satisfied same-engine (DVE-on-DVE) waits
    for k, inst in enumerate(dve):
        si = inst.sync_info
        waits = list(si.on_wait) if si.on_wait else []
        kept = []
        for w in waits:
            if w.sync_type == "semaphore" \
                    and w.ant_name.startswith("DVE") \
                    and w.wait_mode == "sem-ge-imm":
                p = pos_of(w.wait_value)
                if cum[p] >= w.wait_value and k - p >= 3:
                    continue  # in-order 2-deep pipeline: always satisfied
            kept.append(w)
        if len(kept) != len(waits):
            si.on_wait = kept

    # pass 2: strip increments nobody waits on, renumber remaining waits.
    # Engine sem updates must stay +1 (walrus asserts UpdateValue==1), so
    # keep an inc only at positions some wait targets (plus the final
    # one) and rewrite every DVE-sem wait value to the new numbering.
    dve_waits = []
    for inst in all_insts:
        si = inst.sync_info
        if si is None or not si.on_wait:
            continue
        for w in si.on_wait:
            if w.sync_type == "semaphore" \
                    and w.ant_name.startswith("DVE"):
                if w.wait_mode != "sem-ge-imm":
                    return  # unknown wait form: keep everything
                dve_waits.append(w)
    keep = {pos_of(w.wait_value) for w in dve_waits}
    last_inc = max(j for j, i in enumerate(dve)
                   if dve_sem_inc(i) is not None)
    keep.add(last_inc)
    newcum = []
    c2 = 0
    for j, inst in enumerate(dve):
        upd = dve_sem_inc(inst)
        if upd is not None:
            if j in keep:
                c2 += 1
            else:
                inst.sync_info.on_update = []
        newcum.append(c2)
    for w in dve_waits:
        w.wait_value = newcum[pos_of(w.wait_value)]


def build():
    nc = bass.Bass("TRN2", target_bir_lowering=False, debug=False)
    x = nc.dram_tensor("x", [ROWS_PER_CORE, E], mybir.dt.float32,
                       kind="ExternalInput")
    out = nc.dram_tensor("out", [ROWS_PER_CORE, E], mybir.dt.float32,
                         kind="ExternalOutput")
    xap = x.ap()
    oap = out.ap()
    f32 = mybir.dt.float32
    with tile.TileContext(nc) as tc:
        with ExitStack() as ctx:
            xpool = ctx.enter_context(tc.tile_pool(name="x", bufs=8))
            ypool = ctx.enter_context(tc.tile_pool(name="y", bufs=8))
            zpool = ctx.enter_context(tc.tile_pool(name="z", bufs=8))
            opool = ctx.enter_context(tc.tile_pool(name="o", bufs=10))
            vpool = ctx.enter_context(tc.tile_pool(name="v8", bufs=8))
            for b in range(NBLOCKS):
                r0 = b * ROWS_PER_BLOCK
                src = xap[r0:r0 + ROWS_PER_BLOCK, :].rearrange(
                    "(p r) e -> p (r e)", p=128)
                dst = oap[r0:r0 + ROWS_PER_BLOCK, :].rearrange(
                    "(p r) e -> p (r e)", p=128)
                xt = xpool.tile([128, BLOCK_FREE], f32)
                if b == 0:
                    # HWDGE half-loads: no Q7 cold start, DVE can begin
                    # on the first half while the second streams in.
                    nc.sync.dma_start(xt[:, :HALF], src[:, :HALF])
                    nc.sync.dma_start(xt[:, HALF:], src[:, HALF:])
                else:
                    nc.gpsimd.dma_start(xt[:], src)
                # epilogue granularity: quarters for the last block to
                # shorten the drain chain, halves otherwise
                nparts = 4 if b == NBLOCKS - 1 else 2
                pw = BLOCK_FREE // nparts        # columns per part
                segs = pw // E                   # segments per part
                for h in range(nparts):
                    h0 = h * pw
                    yt = ypool.tile([128, pw], f32, tag="y")
                    v8 = vpool.tile([128, 8 * segs], f32, tag="v8")
                    for s in range(segs):
                        seg = slice(h0 + s * E, h0 + (s + 1) * E)
                        nc.vector.max(v8[:, s * 8:(s + 1) * 8], xt[:, seg])
                    for s in range(segs):
                        seg = slice(h0 + s * E, h0 + (s + 1) * E)
                        nc.vector.match_replace(
                            yt[:, s * E:(s + 1) * E], v8[:, s * 8:(s + 1) * 8],
                            xt[:, seg], BETA)
                    zt = zpool.tile([128, pw], f32, tag="z")
                    nc.scalar.activation(zt[:], yt[:],
                                         mybir.ActivationFunctionType.Copy,
                                         bias=BETA, scale=-1.0)
                    ot = opool.tile([128, pw], f32, tag="o")
                    nc.gpsimd.tensor_tensor(ot[:], zt[:], xt[:, h0:h0 + pw],
                                            op=mybir.AluOpType.add)
                    nc.sync.dma_start(dst[:, h0:h0 + pw], ot[:])
    optimize_dve_sems(nc)
    split_sync_waits(nc)
    return nc


_nc_cache = None


def _get_nc():
    global _nc_cache
    if _nc_cache is None:
        _nc_cache = build()
    return _nc_cache


def kernel(x: np.ndarray, _trace: bool = False, **_trace_kwargs):
    x = np.ascontiguousarray(np.asarray(x, dtype=np.float32))
    assert x.shape == (N, E), x.shape
    nc = _get_nc()
    in_maps = [
        {"x": x[c * ROWS_PER_CORE:(c + 1) * ROWS_PER_CORE]}
        for c in range(NCORES)
    ]
    res = run_bass_kernel_spmd(nc, in_maps, core_ids=list(range(NCORES)),
                               trace=_trace, **_trace_kwargs)
    out = np.concatenate([res.results[c]["out"] for c in range(NCORES)],
                         axis=0)
    if _trace:
        return out, res
    return out


# revision 37
# speedup vs baseline: 1.0092x; 1.0011x over previous
"""KeepTopK kernel for Trainium2.

out[i, j] = x[i, j] if x[i, j] is among the top-8 of row i else 1e6.

Strategy (pure data parallel, 8 cores, 32768 rows each):
  per [128, 2048] block (1024 rows, 8 rows per partition):
    load  : 1MB SWDGE loads (nc.gpsimd) — SWDGE completions use the
            DMASW semaphore lanes, disjoint from the stores' DMAHW
            lanes, so a slow store receipt can never false-block a
            load consumer.  Block 0 loads as 2x 512KB via HWDGE
            (nc.sync): faster first-byte than the cold Q7 SWDGE path,
            and half-granular so DVE starts ~3us earlier.
    DVE   : per 256-wide row segment: v8 = max8(x_seg) then
            y = match_replace(x, v8, BETA)   (top-8 positions -> BETA)
    ACT   : z = -y + BETA        per half-block (0 at top-8, else ~BETA)
    POOL  : o = z + x            per half-block (exact x at top-8)
    store : per half-block 512KB from SP (qSPDynamicHW).  The last
            block runs its ACT/POOL/store stages per quarter-block to
            shorten the drain chain.
match_replace replaces exactly one occurrence per top-8 element in index
order, matching jax.lax.top_k tie semantics bitwise.
"""
import numpy as np
from contextlib import ExitStack

import concourse.bass as bass
import concourse.mybir as mybir
import concourse.tile as tile
from concourse.bass_utils import run_bass_kernel_spmd

N, E, K = 262144, 256, 8
BETA = 1000000.0
NCORES = 8
ROWS_PER_CORE = N // NCORES          # 32768
ROWS_PER_PART = 8                    # rows packed per SBUF partition
BLOCK_FREE = ROWS_PER_PART * E       # 2048
ROWS_PER_BLOCK = 128 * ROWS_PER_PART  # 1024
NBLOCKS = ROWS_PER_CORE // ROWS_PER_BLOCK  # 32
HALF = BLOCK_FREE // 2               # 1024

MAX_WAITS = 1


def split_sync_waits(nc, max_waits=MAX_WAITS):
    """walrus codegen rejects instructions with more than one embedded sync
    wait; hoist extras onto same-engine NoOps placed immediately before."""
    spill_id = 0
    for f in nc.m.functions:
        for bb in f.blocks:
            insts = list(bb.instructions)
            new_insts = []
            changed = False
            for inst in insts:
                si = inst.sync_info
                waits = list(si.on_wait) if si and si.on_wait else []
                if len(waits) > max_waits:
                    extra = waits[:-max_waits]
                    si.on_wait = waits[-max_waits:]
                    for j in range(0, len(extra), max_waits):
                        nop = mybir.InstNoOp(
                            name=f"waitspill-{spill_id}", ins=[], outs=[])
                        spill_id += 1
                        nop.engine = inst.engine
                        nop.sync_info = type(si)(
                            on_wait=extra[j:j + max_waits], on_update=[])
                        new_insts.append(nop)
                    changed = True
                new_insts.append(inst)
            if changed:
                bb.instructions = new_insts


def optimize_dve_sems(nc):
    """Reduce per-instruction semaphore bookkeeping on the DVE stream.

    1. Drop DVE-on-DVE waits whose producer is >=3 instructions earlier in
       the DVE program order: the DVE pipeline is 2 deep and completes in
       order, so such waits are always satisfied at dispatch.
    2. Merge runs of [DVE-sem +1] updates into one +K on the last
       instruction of each run.  Cross-engine consumers see the increment
       at run end (later but monotonically correct).  Runs break at any
       instruction that still carries a wait, so an intra-run wait can
       never reference an increment deferred past itself.
    """
    all_insts = [i for f in nc.m.functions for bb in f.blocks
                 for i in bb.instructions]
    dve = [i for i in all_insts
           if getattr(i, "engine", None) == mybir.EngineType.DVE
           and i.sync_info is not None]

    def dve_sem_inc(inst):
        si = inst.sync_info
        ups = list(si.on_update) if si and si.on_update else []
        if len(ups) == 1 and ups[0].sync_type == "semaphore" \
                and ups[0].ant_name.startswith("DVE") \
                and ups[0].update_mode == "sem-inc" \
                and ups[0].update_value == 1:
            return ups[0]
        return None

    # cumulative original inc count after each DVE instruction
    cum = []
    c = 0
    for inst in dve:
        if dve_sem_inc(inst) is not None:
            c += 1
        cum.append(c)
    if not dve or c == 0:
        return

    def pos_of(v):
        for j, cj in enumerate(cum):
            if cj >= v:
                return j
        return len(cum) - 1

    # pass 1: drop trivially-satisfied same-engine (DVE-on-DVE) waits
    for k, inst in enumerate(dve):
        si = inst.sync_info
        waits = list(si.on_wait) if si.on_wait else []
        kept = []
        for w in waits:
            if w.sync_type == "semaphore" \
                    and w.ant_name.startswith("DVE") \
                    and w.wait_mode == "sem-ge-imm":
                p = pos_of(w.wait_value)
                if cum[p] >= w.wait_value and k - p >= 3:
                    continue  # in-order 2-deep pipeline: always satisfied
            kept.append(w)
        if len(kept) != len(waits):
            si.on_wait = kept

    # pass 2: strip increments nobody waits on, renumber remaining waits.
    # Engine sem updates must stay +1 (walrus asserts UpdateValue==1), so
    # keep an inc only at positions some wait targets (plus the final
    # one) and rewrite every DVE-sem wait value to the new numbering.
    dve_waits = []
    for inst in all_insts:
        si = inst.sync_info
        if si is None or not si.on_wait:
            continue
        for w in si.on_wait:
            if w.sync_type == "semaphore" \
                    and w.ant_name.startswith("DVE"):
                if w.wait_mode != "sem-ge-imm":
                    return  # unknown wait form: keep everything
                dve_waits.append(w)
    keep = {pos_of(w.wait_value) for w in dve_waits}
    last_inc = max(j for j, i in enumerate(dve)
                   if dve_sem_inc(i) is not None)
    keep.add(last_inc)
    newcum = []
    c2 = 0
    for j, inst in enumerate(dve):
        upd = dve_sem_inc(inst)
        if upd is not None:
            if j in keep:
                c2 += 1
            else:
                inst.sync_info.on_update = []
        newcum.append(c2)
    for w in dve_waits:
        w.wait_value = newcum[pos_of(w.wait_value)]


def build():
    nc = bass.Bass("TRN2", target_bir_lowering=False, debug=False)
    x = nc.dram_tensor("x", [ROWS_PER_CORE, E], mybir.dt.float32,
                       kind="ExternalInput")
    out = nc.dram_tensor("out", [ROWS_PER_CORE, E], mybir.dt.float32,
                         kind="ExternalOutput")
    xap = x.ap()
    oap = out.ap()
    f32 = mybir.dt.float32
    with tile.TileContext(nc) as tc:
        with ExitStack() as ctx:
            xpool = ctx.enter_context(tc.tile_pool(name="x", bufs=8))
            ypool = ctx.enter_context(tc.tile_pool(name="y", bufs=8))
            zpool = ctx.enter_context(tc.tile_pool(name="z", bufs=8))
            opool = ctx.enter_context(tc.tile_pool(name="o", bufs=10))
            vpool = ctx.enter_context(tc.tile_pool(name="v8", bufs=8))
            for b in range(NBLOCKS):
                r0 = b * ROWS_PER_BLOCK
                src = xap[r0:r0 + ROWS_PER_BLOCK, :].rearrange(
                    "(p r) e -> p (r e)", p=128)
                dst = oap[r0:r0 + ROWS_PER_BLOCK, :].rearrange(
                    "(p r) e -> p (r e)", p=128)
                xt = xpool.tile([128, BLOCK_FREE], f32)
                if b == 0:
                    # HWDGE half-loads: no Q7 cold start, DVE can begin
                    # on the first half while the second streams in.
                    nc.sync.dma_start(xt[:, :HALF], src[:, :HALF])
                    nc.sync.dma_start(xt[:, HALF:], src[:, HALF:])
                else:
                    nc.gpsimd.dma_start(xt[:], src)
                # epilogue granularity: quarters for the last block to
                # shorten the drain chain, halves otherwise
                nparts = 4 if b == NBLOCKS - 1 else 2
                pw = BLOCK_FREE // nparts        # columns per part
                segs = pw // E                   # segments per part
                for h in range(nparts):
                    h0 = h * pw
                    yt = ypool.tile([128, pw], f32, tag="y")
                    v8 = vpool.tile([128, 8 * segs], f32, tag="v8")
                    for s in range(segs):
                        seg = slice(h0 + s * E, h0 + (s + 1) * E)
                        v = v8[:, s * 8:(s + 1) * 8]
                        nc.vector.max(v, xt[:, seg])
                        nc.vector.match_replace(
                            yt[:, s * E:(s + 1) * E], v, xt[:, seg], BETA)
                    zt = zpool.tile([128, pw], f32, tag="z")
                    nc.scalar.activation(zt[:], yt[:],
                                         mybir.ActivationFunctionType.Copy,
                                         bias=BETA, scale=-1.0)
                    ot = opool.tile([128, pw], f32, tag="o")
                    nc.gpsimd.tensor_tensor(ot[:], zt[:], xt[:, h0:h0 + pw],
                                            op=mybir.AluOpType.add)
                    nc.sync.dma_start(dst[:, h0:h0 + pw], ot[:])
    split_sync_waits(nc)
    return nc


_nc_cache = None


def _get_nc():
    global _nc_cache
    if _nc_cache is None:
        _nc_cache = build()
    return _nc_cache


def kernel(x: np.ndarray, _trace: bool = False, **_trace_kwargs):
    x = np.ascontiguousarray(np.asarray(x, dtype=np.float32))
    assert x.shape == (N, E), x.shape
    nc = _get_nc()
    in_maps = [
        {"x": x[c * ROWS_PER_CORE:(c + 1) * ROWS_PER_CORE]}
        for c in range(NCORES)
    ]
    res = run_bass_kernel_spmd(nc, in_maps, core_ids=list(range(NCORES)),
                               trace=_trace, **_trace_kwargs)
    out = np.concatenate([res.results[c]["out"] for c in range(NCORES)],
                         axis=0)
    if _trace:
        return out, res
    return out


# revision 38
# speedup vs baseline: 1.0138x; 1.0046x over previous
"""KeepTopK kernel for Trainium2.

out[i, j] = x[i, j] if x[i, j] is among the top-8 of row i else 1e6.

Strategy (pure data parallel, 8 cores, 32768 rows each):
  per [128, 2048] block (1024 rows, 8 rows per partition):
    load  : 1MB SWDGE loads (nc.gpsimd) — SWDGE completions use the
            DMASW semaphore lanes, disjoint from the stores' DMAHW
            lanes, so a slow store receipt can never false-block a
            load consumer.  Block 0 loads as 2x 512KB via HWDGE
            (nc.sync): faster first-byte than the cold Q7 SWDGE path,
            and half-granular so DVE starts ~3us earlier.
    DVE   : per 256-wide row segment: v8 = max8(x_seg) then
            y = match_replace(x, v8, BETA)   (top-8 positions -> BETA)
    ACT   : z = -y + BETA        per half-block (0 at top-8, else ~BETA)
    POOL  : o = z + x            per half-block (exact x at top-8)
    store : per half-block 512KB from SP (qSPDynamicHW).  The last
            block runs its ACT/POOL/store stages per quarter-block to
            shorten the drain chain.
match_replace replaces exactly one occurrence per top-8 element in index
order, matching jax.lax.top_k tie semantics bitwise.
"""
import numpy as np
from contextlib import ExitStack

import concourse.bass as bass
import concourse.mybir as mybir
import concourse.tile as tile
from concourse.bass_utils import run_bass_kernel_spmd

N, E, K = 262144, 256, 8
BETA = 1000000.0
NCORES = 8
ROWS_PER_CORE = N // NCORES          # 32768
ROWS_PER_PART = 8                    # rows packed per SBUF partition
BLOCK_FREE = ROWS_PER_PART * E       # 2048
ROWS_PER_BLOCK = 128 * ROWS_PER_PART  # 1024
NBLOCKS = ROWS_PER_CORE // ROWS_PER_BLOCK  # 32
HALF = BLOCK_FREE // 2               # 1024

MAX_WAITS = 1


def split_sync_waits(nc, max_waits=MAX_WAITS):
    """walrus codegen rejects instructions with more than one embedded sync
    wait; hoist extras onto same-engine NoOps placed immediately before."""
    spill_id = 0
    for f in nc.m.functions:
        for bb in f.blocks:
            insts = list(bb.instructions)
            new_insts = []
            changed = False
            for inst in insts:
                si = inst.sync_info
                waits = list(si.on_wait) if si and si.on_wait else []
                if len(waits) > max_waits:
                    extra = waits[:-max_waits]
                    si.on_wait = waits[-max_waits:]
                    for j in range(0, len(extra), max_waits):
                        nop = mybir.InstNoOp(
                            name=f"waitspill-{spill_id}", ins=[], outs=[])
                        spill_id += 1
                        nop.engine = inst.engine
                        nop.sync_info = type(si)(
                            on_wait=extra[j:j + max_waits], on_update=[])
                        new_insts.append(nop)
                    changed = True
                new_insts.append(inst)
            if changed:
                bb.instructions = new_insts


def optimize_dve_sems(nc):
    """Reduce per-instruction semaphore bookkeeping on the DVE stream.

    1. Drop DVE-on-DVE waits whose producer is >=3 instructions earlier in
       the DVE program order: the DVE pipeline is 2 deep and completes in
       order, so such waits are always satisfied at dispatch.
    2. Merge runs of [DVE-sem +1] updates into one +K on the last
       instruction of each run.  Cross-engine consumers see the increment
       at run end (later but monotonically correct).  Runs break at any
       instruction that still carries a wait, so an intra-run wait can
       never reference an increment deferred past itself.
    """
    all_insts = [i for f in nc.m.functions for bb in f.blocks
                 for i in bb.instructions]
    dve = [i for i in all_insts
           if getattr(i, "engine", None) == mybir.EngineType.DVE
           and i.sync_info is not None]

    def dve_sem_inc(inst):
        si = inst.sync_info
        ups = list(si.on_update) if si and si.on_update else []
        if len(ups) == 1 and ups[0].sync_type == "semaphore" \
                and ups[0].ant_name.startswith("DVE") \
                and ups[0].update_mode == "sem-inc" \
                and ups[0].update_value == 1:
            return ups[0]
        return None

    # cumulative original inc count after each DVE instruction
    cum = []
    c = 0
    for inst in dve:
        if dve_sem_inc(inst) is not None:
            c += 1
        cum.append(c)
    if not dve or c == 0:
        return

    def pos_of(v):
        for j, cj in enumerate(cum):
            if cj >= v:
                return j
        return len(cum) - 1

    # pass 1: drop trivially-satisfied same-engine (DVE-on-DVE) waits
    for k, inst in enumerate(dve):
        si = inst.sync_info
        waits = list(si.on_wait) if si.on_wait else []
        kept = []
        for w in waits:
            if w.sync_type == "semaphore" \
                    and w.ant_name.startswith("DVE") \
                    and w.wait_mode == "sem-ge-imm":
                p = pos_of(w.wait_value)
                if cum[p] >= w.wait_value and k - p >= 3:
                    continue  # in-order 2-deep pipeline: always satisfied
            kept.append(w)
        if len(kept) != len(waits):
            si.on_wait = kept

    # pass 2: strip increments nobody waits on, renumber remaining waits.
    # Engine sem updates must stay +1 (walrus asserts UpdateValue==1), so
    # keep an inc only at positions some wait targets (plus the final
    # one) and rewrite every DVE-sem wait value to the new numbering.
    dve_waits = []
    for inst in all_insts:
        si = inst.sync_info
        if si is None or not si.on_wait:
            continue
        for w in si.on_wait:
            if w.sync_type == "semaphore" \
                    and w.ant_name.startswith("DVE"):
                if w.wait_mode != "sem-ge-imm":
                    return  # unknown wait form: keep everything
                dve_waits.append(w)
    keep = {pos_of(w.wait_value) for w in dve_waits}
    last_inc = max(j for j, i in enumerate(dve)
                   if dve_sem_inc(i) is not None)
    keep.add(last_inc)
    newcum = []
    c2 = 0
    for j, inst in enumerate(dve):
        upd = dve_sem_inc(inst)
        if upd is not None:
            if j in keep:
                c2 += 1
            else:
                inst.sync_info.on_update = []
        newcum.append(c2)
    for w in dve_waits:
        w.wait_value = newcum[pos_of(w.wait_value)]


def build():
    nc = bass.Bass("TRN2", target_bir_lowering=False, debug=False)
    x = nc.dram_tensor("x", [ROWS_PER_CORE, E], mybir.dt.float32,
                       kind="ExternalInput")
    out = nc.dram_tensor("out", [ROWS_PER_CORE, E], mybir.dt.float32,
                         kind="ExternalOutput")
    xap = x.ap()
    oap = out.ap()
    f32 = mybir.dt.float32
    with tile.TileContext(nc) as tc:
        with ExitStack() as ctx:
            xpool = ctx.enter_context(tc.tile_pool(name="x", bufs=8))
            ypool = ctx.enter_context(tc.tile_pool(name="y", bufs=8))
            zpool = ctx.enter_context(tc.tile_pool(name="z", bufs=8))
            opool = ctx.enter_context(tc.tile_pool(name="o", bufs=10))
            vpool = ctx.enter_context(tc.tile_pool(name="v8", bufs=8))
            for b in range(NBLOCKS):
                r0 = b * ROWS_PER_BLOCK
                src = xap[r0:r0 + ROWS_PER_BLOCK, :].rearrange(
                    "(p r) e -> p (r e)", p=128)
                dst = oap[r0:r0 + ROWS_PER_BLOCK, :].rearrange(
                    "(p r) e -> p (r e)", p=128)
                xt = xpool.tile([128, BLOCK_FREE], f32)
                if b < 3:
                    # HWDGE half-loads for the first blocks: no Q7 cold
                    # start, DVE can begin on the first half while the
                    # rest streams in.  All complete before the first
                    # store, so DMAHW lane sharing is harmless here.
                    nc.sync.dma_start(xt[:, :HALF], src[:, :HALF])
                    nc.sync.dma_start(xt[:, HALF:], src[:, HALF:])
                else:
                    nc.gpsimd.dma_start(xt[:], src)
                # epilogue granularity: quarters for the last block to
                # shorten the drain chain, halves otherwise
                nparts = 4 if b == NBLOCKS - 1 else 2
                pw = BLOCK_FREE // nparts        # columns per part
                segs = pw // E                   # segments per part
                for h in range(nparts):
                    h0 = h * pw
                    yt = ypool.tile([128, pw], f32, tag="y")
                    v8 = vpool.tile([128, 8 * segs], f32, tag="v8")
                    for s in range(segs):
                        seg = slice(h0 + s * E, h0 + (s + 1) * E)
                        v = v8[:, s * 8:(s + 1) * 8]
                        nc.vector.max(v, xt[:, seg])
                        nc.vector.match_replace(
                            yt[:, s * E:(s + 1) * E], v, xt[:, seg], BETA)
                    zt = zpool.tile([128, pw], f32, tag="z")
                    nc.scalar.activation(zt[:], yt[:],
                                         mybir.ActivationFunctionType.Copy,
                                         bias=BETA, scale=-1.0)
                    ot = opool.tile([128, pw], f32, tag="o")
                    nc.gpsimd.tensor_tensor(ot[:], zt[:], xt[:, h0:h0 + pw],
                                            op=mybir.AluOpType.add)
                    nc.sync.dma_start(dst[:, h0:h0 + pw], ot[:])
    split_sync_waits(nc)
    return nc


_nc_cache = None


def _get_nc():
    global _nc_cache
    if _nc_cache is None:
        _nc_cache = build()
    return _nc_cache


def kernel(x: np.ndarray, _trace: bool = False, **_trace_kwargs):
    x = np.ascontiguousarray(np.asarray(x, dtype=np.float32))
    assert x.shape == (N, E), x.shape
    nc = _get_nc()
    in_maps = [
        {"x": x[c * ROWS_PER_CORE:(c + 1) * ROWS_PER_CORE]}
        for c in range(NCORES)
    ]
    res = run_bass_kernel_spmd(nc, in_maps, core_ids=list(range(NCORES)),
                               trace=_trace, **_trace_kwargs)
    out = np.concatenate([res.results[c]["out"] for c in range(NCORES)],
                         axis=0)
    if _trace:
        return out, res
    return out
